# revision 30
# baseline (speedup 1.0000x reference)
"""Bahdanau-style attention kernel for Trainium2, 8 NeuronCores.

Reference computation (per batch b):
    score  = tanh(c @ W1 + W1_b + (h @ W2 + W2_b)[None, :])   # [T, U]
    logits = score @ V_w (+ V_b, cancels in softmax)          # [T, 1]
    attn   = softmax(logits over T)
    out    = sum_t attn[t] * c[t, :]                          # [D]

Sharding: pure data-parallel over batch B=64 across 8 cores (8 batches/core).
No collectives; host concatenates per-core outputs.

Host-side marshalling: c is cast to bf16 and shipped transposed [BL, D, T]
(the only layout the kernel needs). All FLOPs run on device.

Per-core dataflow ([u,t] orientation), per batch (T=2048 = 4 t-supers of 512):
  - one 2MB DMA for cT [d128, (k t)]
  - main matmul on TensorE: psum_uc[u128, t512] += W1_chunk.T @ cT_chunk (bf16)
  - tanh on ScalarE with per-partition bias = (h@W2 + b)[u-chunk] -> score^T bf16
  - V-dot on TensorE: psum_v[1, t512] += V_chunk.T @ score^T_chunk
  - exp on ScalarE -> w_row bf16, accum_out -> per-super softmax denominator
  - broadcast w to 128 partitions via K=1 ones-matmul + DVE copy
  - pass-2 on VectorE: ctx[d-chunk, 1] = reduce_t(cT_chunk * w_bcast)
  - per-batch tail: reduce supers, transpose [128,4]->[4,128] on TensorE,
    divide by denominator on DVE, DMA out.
"""

import ml_dtypes
import numpy as np

import concourse.bass as bass
import concourse.tile as tile
from concourse import bacc, mybir
from concourse import bass_utils

B, T, D, U = 64, 2048, 512, 512
NCORES = 8
BL = B // NCORES  # 8 batches per core
KD = D // 128     # 4 contraction chunks
NST = T // 512    # 4 t-supers per batch
F32 = mybir.dt.float32
BF16 = mybir.dt.bfloat16
AF = mybir.ActivationFunctionType
ALU = mybir.AluOpType


def build_nc(n_batch=BL, repeat=1, stage=7, psp_bufs=5, psv_bufs=2, ct_bufs=2,
             score_bufs=12, work_bufs=6, sync_load=True):
    # stage: 1=loads 2=+main-mms 3=+tanh 4=+vdot 5=+exp+bcast 6=+pass2 7=full
    nc = bacc.Bacc(None, target_bir_lowering=False)

    ct_ext = nc.declare_dram_parameter("ct", [BL, D, T], BF16, isOutput=False)
    h_ext = nc.declare_dram_parameter("h", [BL, D], F32, isOutput=False)
    w1_ext = nc.declare_dram_parameter("W1_w", [D, U], F32, isOutput=False)
    b1_ext = nc.declare_dram_parameter("W1_b", [U], F32, isOutput=False)
    w2_ext = nc.declare_dram_parameter("W2_w", [D, U], F32, isOutput=False)
    b2_ext = nc.declare_dram_parameter("W2_b", [U], F32, isOutput=False)
    v_ext = nc.declare_dram_parameter("V_w", [U, 1], F32, isOutput=False)
    ones_ext = nc.declare_dram_parameter("ones", [128, 128], F32, isOutput=False)
    eye_ext = nc.declare_dram_parameter("eye", [128, 128], F32, isOutput=False)
    out_ext = nc.declare_dram_parameter("out", [BL, D], F32, isOutput=True)

    with tile.TileContext(nc) as tc:
        with (
            tc.tile_pool(name="const", bufs=1) as constp,
            tc.tile_pool(name="ct", bufs=ct_bufs) as ctp,
            tc.tile_pool(name="work", bufs=work_bufs) as workp,
            tc.tile_pool(name="score", bufs=score_bufs) as scorep,
        ):
            # ---------------- setup (one-time) ----------------
            with tc.tile_pool(name="spsum", bufs=1, space="PSUM") as sps:
                ones_f = constp.tile([128, 128], F32)
                nc.gpsimd.dma_start(ones_f[:], ones_ext[:, :])
                ones_bf = constp.tile([128, 128], BF16)
                nc.scalar.activation(ones_bf[:], ones_f[:], AF.Copy)
                eye_f = constp.tile([128, 128], F32)
                nc.gpsimd.dma_start(eye_f[:], eye_ext[:, :])
                eye_bf = constp.tile([128, 128], BF16)
                nc.scalar.activation(eye_bf[:], eye_f[:], AF.Copy)

                # W1 chunks [d128, (k u)] bf16: lhsT slice [d, u-chunk]
                w1_f = constp.tile([128, KD * U], F32)
                nc.gpsimd.dma_start(
                    w1_f[:].rearrange("p (k u) -> p k u", k=KD),
                    w1_ext.rearrange("(k p) u -> p k u", p=128),
                )
                w1_bf = constp.tile([128, KD * U], BF16)
                nc.vector.tensor_copy(w1_bf[:], w1_f[:])

                w2_f = constp.tile([128, KD * U], F32)
                nc.gpsimd.dma_start(
                    w2_f[:].rearrange("p (k u) -> p k u", k=KD),
                    w2_ext.rearrange("(k p) u -> p k u", p=128),
                )
                w2_bf = constp.tile([128, KD * U], BF16)
                nc.vector.tensor_copy(w2_bf[:], w2_f[:])

                # h [BL, D] -> hT [d128, (k 16)] bf16 via DMA transpose
                h_f = constp.tile([16, D], F32)
                nc.vector.memset(h_f[:], 0.0)
                nc.gpsimd.dma_start(h_f[0:BL, :], h_ext[:, :])
                h_bf = constp.tile([16, D], BF16)
                nc.vector.tensor_copy(h_bf[:], h_f[:])
                hT_bf = constp.tile([128, KD * 16], BF16)
                for k in range(KD):
                    nc.sync.dma_start(
                        out=hT_bf[:, 16 * k : 16 * (k + 1)],
                        in_=h_bf[0:16, 128 * k : 128 * (k + 1)],
                        transpose=True,
                    )

                b1_f = constp.tile([1, U], F32)
                nc.gpsimd.dma_start(b1_f[:], b1_ext[None, :])
                b2_f = constp.tile([1, U], F32)
                nc.gpsimd.dma_start(b2_f[:], b2_ext[None, :])
                b12_f = constp.tile([1, U], F32)
                nc.vector.tensor_add(b12_f[:], b1_f[:], b2_f[:])
                b12_bf = constp.tile([1, U], BF16)
                nc.scalar.activation(b12_bf[:], b12_f[:], AF.Copy)

                # hb[b, u] = h[b] @ W2 + W1_b + W2_b   (rows 0:BL valid)
                ps_hb = sps.tile([16, U], F32)
                for k in range(KD):
                    nc.tensor.matmul(
                        ps_hb[:],
                        lhsT=hT_bf[:, 16 * k : 16 * (k + 1)],
                        rhs=w2_bf[:, U * k : U * (k + 1)],
                        start=(k == 0),
                        stop=False,
                    )
                nc.tensor.matmul(
                    ps_hb[:], lhsT=ones_bf[0:1, 0:16], rhs=b12_bf[:],
                    start=False, stop=True,
                )
                hbr_f = constp.tile([16, U], F32)
                nc.scalar.activation(hbr_f[:], ps_hb[:], AF.Copy)
                # transpose to hbT [u128, (k 16)] f32 (tanh bias columns)
                ps_hbt = sps.tile([128, KD * 16], F32)
                for k in range(KD):
                    nc.tensor.transpose(
                        ps_hbt[:, 16 * k : 16 * (k + 1)],
                        hbr_f[0:16, 128 * k : 128 * (k + 1)],
                        eye_f[0:16, 0:16],
                    )
                hbT_f = constp.tile([128, KD * 16], F32)
                nc.scalar.activation(hbT_f[:], ps_hbt[:], AF.Copy)

                # V replicated: vrep_k [u128, 128] bf16, every column = V[u-chunk k]
                v_f = constp.tile([1, U], F32)
                nc.gpsimd.dma_start(v_f[:], v_ext.rearrange("u o -> o u"))
                v_bf = constp.tile([1, U], BF16)
                nc.scalar.activation(v_bf[:], v_f[:], AF.Copy)
                vcol_f = constp.tile([128, KD], F32)
                for k in range(KD):
                    ps_v = sps.tile([128, 1], F32, tag=f"psv{k}")
                    nc.tensor.transpose(
                        ps_v[:], v_f[0:1, 128 * k : 128 * (k + 1)], eye_f[0:1, 0:1]
                    )
                    nc.scalar.activation(vcol_f[:, k : k + 1], ps_v[:], AF.Copy)
                vrep_bf = constp.tile([128, KD * 128], BF16)
                for k in range(KD):
                    nc.vector.tensor_scalar(
                        out=vrep_bf[:, 128 * k : 128 * (k + 1)],
                        in0=ones_f[:, :],
                        scalar1=0.0,
                        scalar2=vcol_f[:, k : k + 1],
                        op0=ALU.mult,
                        op1=ALU.add,
                    )

            # ---------------- main loop ----------------
            with (
                tc.tile_pool(name="psum_s", bufs=psp_bufs, space="PSUM") as psp,
                tc.tile_pool(name="psum_v", bufs=psv_bufs, space="PSUM") as psvp,
                tc.tile_pool(name="psum_t", bufs=1, space="PSUM") as ptp,
            ):
                for rep in range(repeat):
                  for b in range(n_batch):
                    cts = ctp.tile([128, KD * T], BF16)
                    load_eng = nc.sync if sync_load else nc.gpsimd
                    load_eng.dma_start(
                        cts[:].rearrange("p (k t) -> p k t", k=KD),
                        ct_ext[b].rearrange("(k p) t -> p k t", p=128),
                    )
                    if stage < 2:
                        continue
                    ctx_all = workp.tile([128, KD * NST], F32, tag="ctxall")
                    s_all = workp.tile([128, NST], F32, tag="sall")
                    for st in range(NST):
                        t0 = 512 * st
                        pss = []
                        for uc in range(KD):
                            ps = psp.tile([128, 512], F32, tag="ps")
                            pss.append(ps)
                            for k in range(KD):
                                nc.tensor.matmul(
                                    ps[:],
                                    lhsT=w1_bf[:, U * k + 128 * uc : U * k + 128 * (uc + 1)],
                                    rhs=cts[:, T * k + t0 : T * k + t0 + 512],
                                    start=(k == 0),
                                    stop=(k == KD - 1),
                                )
                        if stage < 3:
                            continue
                        scs = []
                        for uc in range(KD):
                            score = scorep.tile([128, 512], BF16, tag="score")
                            scs.append(score)
                            nc.scalar.activation(
                                score[:], pss[uc][:], AF.Tanh,
                                bias=hbT_f[:, 16 * uc + b : 16 * uc + b + 1],
                            )
                        if stage < 4:
                            continue
                        # logits broadcast across partitions via replicated-V
                        psv = psvp.tile([128, 512], F32, tag="psv")
                        for uc in range(KD):
                            nc.tensor.matmul(
                                psv[:],
                                lhsT=vrep_bf[:, 128 * uc : 128 * (uc + 1)],
                                rhs=scs[uc][:],
                                start=(uc == 0),
                                stop=(uc == KD - 1),
                            )
                        if stage < 5:
                            continue
                        wb = workp.tile([128, 512], BF16, tag="wb")
                        nc.scalar.activation(
                            wb[:], psv[:], AF.Exp,
                            accum_out=s_all[:, st : st + 1],
                        )
                        if stage < 6:
                            continue
                        for k in range(KD):
                            prod2 = workp.tile([128, 512], BF16, tag="prod2")
                            nc.vector.tensor_mul(
                                prod2[:], cts[:, T * k + t0 : T * k + t0 + 512], wb[:]
                            )
                            nc.vector.reduce_sum(
                                ctx_all[:, NST * k + st : NST * k + st + 1],
                                prod2[:],
                                axis=mybir.AxisListType.X,
                            )
                    if stage < 7:
                        continue
                    # ---- batch tail ----
                    stot = workp.tile([128, 1], F32, tag="stot")
                    nc.vector.reduce_sum(stot[:], s_all[:], axis=mybir.AxisListType.X)
                    invc = workp.tile([128, 1], F32, tag="invc")
                    nc.vector.reciprocal(invc[:], stot[:])
                    ctxs = workp.tile([128, KD], F32, tag="ctxs")
                    for k in range(KD):
                        nc.vector.reduce_sum(
                            ctxs[:, k : k + 1],
                            ctx_all[:, NST * k : NST * (k + 1)],
                            axis=mybir.AxisListType.X,
                        )
                    ctxn = workp.tile([128, KD], F32, tag="ctxn")
                    nc.vector.tensor_scalar_mul(ctxn[:], ctxs[:], invc[:, 0:1])
                    pst = ptp.tile([4, 128], F32, tag="pst")
                    nc.tensor.transpose(pst[:], ctxn[:], eye_f[:, :])
                    orow4 = workp.tile([4, 128], F32, tag="orow4")
                    nc.scalar.activation(orow4[:], pst[:], AF.Copy)
                    nc.gpsimd.dma_start(
                        out_ext[b].rearrange("(k f) -> k f", k=KD), orow4[:]
                    )
    nc.compile()
    return nc


_NC_CACHE = None


def _get_nc():
    global _NC_CACHE
    if _NC_CACHE is None:
        _NC_CACHE = build_nc()
    return _NC_CACHE


def make_in_maps(c, h, W1_w, W1_b, W2_w, W2_b, V_w):
    c = np.asarray(c, np.float32)
    cb = c.astype(ml_dtypes.bfloat16)                    # [B, T, D] bf16
    ct = np.ascontiguousarray(cb.swapaxes(1, 2))         # [B, D, T] bf16
    shared = {
        "W1_w": np.ascontiguousarray(np.asarray(W1_w, np.float32)),
        "W1_b": np.ascontiguousarray(np.asarray(W1_b, np.float32)),
        "W2_w": np.ascontiguousarray(np.asarray(W2_w, np.float32)),
        "W2_b": np.ascontiguousarray(np.asarray(W2_b, np.float32)),
        "V_w": np.ascontiguousarray(np.asarray(V_w, np.float32)),
        "ones": np.ones((128, 128), np.float32),
        "eye": np.eye(128, dtype=np.float32),
    }
    h = np.asarray(h, np.float32)
    in_maps = []
    for i in range(NCORES):
        m = dict(shared)
        m["ct"] = ct[i * BL : (i + 1) * BL]
        m["h"] = np.ascontiguousarray(h[i * BL : (i + 1) * BL])
        in_maps.append(m)
    return in_maps


def kernel(**inputs):
    in_maps = make_in_maps(
        inputs["c"], inputs["h"], inputs["W1_w"], inputs["W1_b"],
        inputs["W2_w"], inputs["W2_b"], inputs["V_w"],
    )
    nc = _get_nc()
    res = bass_utils.run_bass_kernel_spmd(nc, in_maps, core_ids=list(range(NCORES)))
    out = np.concatenate([np.asarray(r["out"]) for r in res.results], axis=0)
    return out.astype(np.float32)


if __name__ == "__main__":
    rng = np.random.default_rng(0)
    ins = {
        "c": rng.standard_normal((B, T, D), dtype=np.float32),
        "h": rng.standard_normal((B, D), dtype=np.float32),
        "W1_w": rng.standard_normal((D, U), dtype=np.float32) / np.sqrt(D),
        "W1_b": np.zeros((U,), np.float32),
        "W2_w": rng.standard_normal((D, U), dtype=np.float32) / np.sqrt(D),
        "W2_b": np.zeros((U,), np.float32),
        "V_w": rng.standard_normal((U, 1), dtype=np.float32) / np.sqrt(U),
        "V_b": np.zeros((1,), np.float32),
    }
    out = kernel(**ins)
    print("out", out.shape, out.dtype, np.abs(out).mean())
